# revision 1
# baseline (speedup 1.0000x reference)
"""DenoiseLSTM Trainium2 kernel (8 NeuronCores, SPMD).

Strategy: the recurrent parts (bi-LSTM encoder, LSTM decoder) are replicated
on all 8 cores (they are latency-bound and do not parallelize profitably);
the dominant vocab projection [B*T,512]@[512,32000] is sharded over V
(4000 columns per core). No collectives; the host concatenates V-shards.

All recurrences run in transposed layout (h.T chunks [128, B]) so the LSTM
elementwise uses all 128 partitions and h.T is directly the next step's
matmul rhs. Input projections x@Wih.T are batched up front into DRAM and
streamed per step. Attention + FFN + vocab projection are batched after the
decoder scan (they do not feed the recurrence).
"""
import sys

sys.path.insert(0, "/opt/trn_rl_repo")

from contextlib import ExitStack

import numpy as np
import ml_dtypes

import concourse.bass as bass
import concourse.bacc as bacc
import concourse.mybir as mybir
import concourse.tile as tile
from concourse.bass_utils import run_bass_kernel_spmd
from concourse.masks import make_identity

bf16 = ml_dtypes.bfloat16
F32 = mybir.dt.float32
BF16 = mybir.dt.bfloat16
I16 = mybir.dt.int16
AF = mybir.ActivationFunctionType
ALU = mybir.AluOpType
AX = mybir.AxisListType

B = 32
D_EMB = 128
D_ENC = 256
D_DEC = 512
N_CORES = 8


class _Stop(Exception):
    pass


def build(S=128, T=128, V=32000, VS=4000, phases=6):
    """Builds + compiles the Bacc module. Returns nc."""
    NI_E = B * S          # encoder gather count
    NI_D = B * T          # decoder gather count
    assert NI_E % 128 == 0 and NI_D % 128 == 0
    assert (B * S) % 512 == 0 and (B * T) % 512 == 0
    KD = D_DEC // 128     # 4 hidden chunks (decoder)
    KE = D_ENC // 128     # 2 hidden chunks (encoder)
    MD = 4 * D_DEC // 128  # 16 gate chunks (decoder)
    ME = 4 * D_ENC // 128  # 8 gate chunks (encoder)
    NBLK_E = (B * S) // 512
    NBLK_D = (B * T) // 512
    NVC = (VS + 499) // 500  # vocab column chunks per core
    SCALE = 1.0 / float(np.sqrt(np.float32(2 * D_ENC)))

    nc = bacc.Bacc("TRN2", target_bir_lowering=False, debug=False)

    # ---- external inputs (host-prepped layouts) ----
    tokb = nc.dram_tensor("tokb", [V, D_EMB], BF16, kind="ExternalInput")
    idx_e = nc.dram_tensor("idx_e", [128, NI_E // 16], I16, kind="ExternalInput")
    idx_d = nc.dram_tensor("idx_d", [128, NI_D // 16], I16, kind="ExternalInput")
    startT = nc.dram_tensor("startT", [128, 1], BF16, kind="ExternalInput")
    # encoder init: h0 = e0 + lab*(e1-e0), per direction (fwd = cols 0:256, bwd 256:512)
    diff_e = nc.dram_tensor("diff_e", [1, 2 * D_ENC], BF16, kind="ExternalInput")
    e0T = nc.dram_tensor("e0T", [128, KD], F32, kind="ExternalInput")
    lab_i = nc.dram_tensor("lab_i", [1, B], BF16, kind="ExternalInput")
    # decoder init h_t from style_emb
    diff_s = nc.dram_tensor("diff_s", [1, D_DEC], BF16, kind="ExternalInput")
    s0T = nc.dram_tensor("s0T", [128, KD], F32, kind="ExternalInput")
    lab_d = nc.dram_tensor("lab_d", [1, B], BF16, kind="ExternalInput")
    # weights, PE-ready (lhsT tiles chained along free dim, (k, m)-major)
    wih_f = nc.dram_tensor("wih_f", [128, ME * 128], BF16, kind="ExternalInput")
    wih_b = nc.dram_tensor("wih_b", [128, ME * 128], BF16, kind="ExternalInput")
    wih_d = nc.dram_tensor("wih_d", [128, MD * 128], BF16, kind="ExternalInput")
    whh_f = nc.dram_tensor("whh_f", [128, KE * ME * 128], BF16, kind="ExternalInput")
    whh_b = nc.dram_tensor("whh_b", [128, KE * ME * 128], BF16, kind="ExternalInput")
    whh_d = nc.dram_tensor("whh_d", [128, KD * MD * 128], BF16, kind="ExternalInput")
    wtr = nc.dram_tensor("wtr", [128, KD * KD * 128], BF16, kind="ExternalInput")
    wf1 = nc.dram_tensor("wf1", [128, 8 * KD * 128], BF16, kind="ExternalInput")
    wf2 = nc.dram_tensor("wf2", [128, KD * VS], BF16, kind="ExternalInput")
    # bias columns (bih+bhh summed on host); b1h = 0.55 * b_f1
    bs_f = nc.dram_tensor("bs_f", [128, ME], F32, kind="ExternalInput")
    bs_b = nc.dram_tensor("bs_b", [128, ME], F32, kind="ExternalInput")
    bs_d = nc.dram_tensor("bs_d", [128, MD], F32, kind="ExternalInput")
    b1a = nc.dram_tensor("b1a", [128, KD], F32, kind="ExternalInput")
    b1h = nc.dram_tensor("b1h", [128, KD], F32, kind="ExternalInput")

    # ---- outputs ----
    out = nc.dram_tensor("out", [B, T, VS], F32, kind="ExternalOutput")

    # ---- DRAM scratch ----
    xf_d = nc.dram_tensor("xf_d", [S, 128, ME * 32], BF16)
    xb_d = nc.dram_tensor("xb_d", [S, 128, ME * 32], BF16)
    xd_d = nc.dram_tensor("xd_d", [T, 128, MD * 32], BF16)

    with tile.TileContext(nc) as tc, ExitStack() as ctx:
        wpool = ctx.enter_context(tc.tile_pool(name="weights", bufs=1))
        spool = ctx.enter_context(tc.tile_pool(name="state", bufs=1))
        big = ctx.enter_context(tc.tile_pool(name="big", bufs=1))

        # ---------- load weights / constants ----------
        def load(dram, shape, dtype, tag):
            t = wpool.tile(shape, dtype, tag=tag, name=tag)
            nc.sync.dma_start(t[:], dram[:, :])
            return t

        wih_f_s = load(wih_f, [128, ME * 128], BF16, "wih_f")
        wih_b_s = load(wih_b, [128, ME * 128], BF16, "wih_b")
        wih_d_s = load(wih_d, [128, MD * 128], BF16, "wih_d")
        whh_f_s = load(whh_f, [128, KE * ME * 128], BF16, "whh_f")
        whh_b_s = load(whh_b, [128, KE * ME * 128], BF16, "whh_b")
        whh_d_s = load(whh_d, [128, KD * MD * 128], BF16, "whh_d")
        wtr_s = load(wtr, [128, KD * KD * 128], BF16, "wtr")
        wf1_s = load(wf1, [128, 8 * KD * 128], BF16, "wf1")
        bs_f_s = load(bs_f, [128, ME], F32, "bs_f")
        bs_b_s = load(bs_b, [128, ME], F32, "bs_b")
        bs_d_s = load(bs_d, [128, MD], F32, "bs_d")
        b1a_s = load(b1a, [128, KD], F32, "b1a")
        b1h_s = load(b1h, [128, KD], F32, "b1h")
        startT_s = load(startT, [128, 1], BF16, "startT")
        e0T_s = load(e0T, [128, KD], F32, "e0T")
        s0T_s = load(s0T, [128, KD], F32, "s0T")
        ident = wpool.tile([128, 128], BF16, tag="ident", name="ident")
        make_identity(nc, ident)

        diff_e_s = wpool.tile([1, 2 * D_ENC], BF16, tag="diff_e", name="diff_e")
        nc.sync.dma_start(diff_e_s[:], diff_e[:, :])
        diff_s_s = wpool.tile([1, D_DEC], BF16, tag="diff_s", name="diff_s")
        nc.sync.dma_start(diff_s_s[:], diff_s[:, :])
        lab_i_s = wpool.tile([1, B], BF16, tag="lab_i", name="lab_i")
        nc.sync.dma_start(lab_i_s[:], lab_i[:, :])
        lab_d_s = wpool.tile([1, B], BF16, tag="lab_d", name="lab_d")
        nc.sync.dma_start(lab_d_s[:], lab_d[:, :])

        # ---------- gathers ----------
        idx_e_s = wpool.tile([128, NI_E // 16], I16, tag="idx_e", name="idx_e")
        nc.sync.dma_start(idx_e_s[:], idx_e[:, :])
        idx_d_s = wpool.tile([128, NI_D // 16], I16, tag="idx_d", name="idx_d")
        nc.sync.dma_start(idx_d_s[:], idx_d[:, :])
        encT = big.tile([128, 1, NI_E], BF16, tag="encT", name="encT")   # cols s*32+b
        nc.gpsimd.dma_gather(encT[:], tokb[:, :], idx_e_s[:], NI_E, NI_E,
                             D_EMB, transpose=True, single_packet=False)
        decT = big.tile([128, 1, NI_D], BF16, tag="decT", name="decT")   # cols t*32+b
        nc.gpsimd.dma_gather(decT[:], tokb[:, :], idx_d_s[:], NI_D, NI_D,
                             D_EMB, transpose=True, single_packet=False)
        # decoder step 0 input = start_emb (broadcast over b)
        nc.vector.tensor_copy(decT[:, 0, 0:B],
                              startT_s[:, 0:1].to_broadcast((128, B)))

        # ---------- input projections -> DRAM ----------
        with tc.tile_pool(name="xp_ps", bufs=4, space="PSUM") as xps, \
             tc.tile_pool(name="xp_sb", bufs=4) as xsb:
            def proj(w_s, src, nblk, nm, bias_s, dst, L):
                # dst [L, 128, nm*32]; block covers 512/B steps
                spb = 512 // B
                for blk in range(nblk):
                    for m in range(nm):
                        ps = xps.tile([128, 512], F32, tag="xp", name="xp")
                        nc.tensor.matmul(ps[:], w_s[:, m * 128:(m + 1) * 128],
                                         src[:, 0, blk * 512:(blk + 1) * 512],
                                         start=True, stop=True)
                        sb = xsb.tile([128, 512], BF16, tag="xs", name="xs")
                        nc.scalar.activation(sb[:], ps[:], AF.Identity,
                                             bias=bias_s[:, m:m + 1])
                        dst_ap = dst.ap()[blk * spb:(blk + 1) * spb, :,
                                          m * 32:(m + 1) * 32]
                        nc.sync.dma_start(
                            dst_ap.rearrange("s p b -> p s b"),
                            sb[:].rearrange("p (s b) -> p s b", b=B))

            proj(wih_f_s, encT, NBLK_E, ME, bs_f_s, xf_d, S)
            proj(wih_b_s, encT, NBLK_E, ME, bs_b_s, xb_d, S)
            proj(wih_d_s, decT, NBLK_D, MD, bs_d_s, xd_d, T)

        try:
            if phases < 2:
                raise _Stop
            # ---------- init states ----------
            # encoder h0 per direction via outer product + broadcast add
            h_f = [spool.tile([128, KE * B], BF16, tag=f"h_f{j}", name=f"h_f{j}") for j in range(2)]
            c_f = spool.tile([128, KE * B], F32, tag="c_f", name="c_f")
            h_b = [spool.tile([128, KE * B], BF16, tag=f"h_b{j}", name=f"h_b{j}") for j in range(2)]
            c_b = spool.tile([128, KE * B], F32, tag="c_b", name="c_b")
            mem_T = big.tile([128, 2 * KE * B * S], BF16, tag="mem_T", name="mem_T")  # cols k*(B*S)+b*S+s

            with tc.tile_pool(name="init_ps", bufs=2, space="PSUM") as ips:
                for d, (hst, dbase) in enumerate([(h_f, 0), (h_b, KE)]):
                    for k in range(KE):
                        ps = ips.tile([128, B], F32, tag="i", name="i")
                        col = (dbase + k) * 128
                        nc.tensor.matmul(ps[:], diff_e_s[:, col:col + 128],
                                         lab_i_s[:, :], start=True, stop=True)
                        f32t = spool.tile([128, B], F32, tag="h0f", name="h0f")
                        nc.vector.tensor_scalar_add(f32t[:], ps[:],
                                                    e0T_s[:, dbase + k:dbase + k + 1])
                        nc.vector.tensor_copy(hst[0][:, k * B:(k + 1) * B], f32t[:])
                nc.vector.memset(c_f[:], 0.0)
                nc.vector.memset(c_b[:], 0.0)

            # ---------- encoder recurrence (both directions interleaved) ----------
            memT4 = mem_T[:].rearrange("p (k b s) -> p k b s", k=2 * KE, b=B)
            with tc.tile_pool(name="enc_ps", bufs=2, space="PSUM") as eps, \
                 tc.tile_pool(name="enc_x", bufs=4) as exp_, \
                 tc.tile_pool(name="enc_g", bufs=3) as egp:
                for step in range(S):
                    for d, (hst, cst, whh_s, xdram, bs_s, kk0) in enumerate([
                            (h_f, c_f, whh_f_s, xf_d, bs_f_s, 0),
                            (h_b, c_b, whh_b_s, xb_d, bs_b_s, KE)]):
                        s_in = step if d == 0 else S - 1 - step
                        s_mem = s_in
                        hcur = hst[step % 2]
                        hnxt = hst[(step + 1) % 2]
                        xt = exp_.tile([128, ME * 32], BF16, tag=f"x{d}", name=f"x{d}")
                        nc.sync.dma_start(xt[:], xdram[s_in, :, :])
                        ps = eps.tile([128, ME * 32], F32, tag=f"g{d}", name=f"g{d}")
                        for m in range(ME):
                            for k in range(KE):
                                rhs = hcur[:, k * B:(k + 1) * B]
                                lt = whh_s[:, (k * ME + m) * 128:(k * ME + m + 1) * 128]
                                nc.tensor.matmul(ps[:, m * 32:(m + 1) * 32], lt, rhs,
                                                 start=(k == 0), stop=(k == KE - 1))
                        g = egp.tile([128, ME * 32], F32, tag=f"gg{d}", name=f"gg{d}")
                        nc.vector.tensor_tensor(g[:], ps[:], xt[:], ALU.add)
                        # gates: i [0:2B*KE], f, g, o   (ME*32 = 4*KE*32)
                        GW = KE * 32  # width of one gate group
                        nc.scalar.activation(g[:, 0:2 * GW], g[:, 0:2 * GW], AF.Sigmoid)
                        nc.scalar.activation(g[:, 2 * GW:3 * GW], g[:, 2 * GW:3 * GW],
                                             AF.Tanh)
                        nc.scalar.activation(g[:, 3 * GW:4 * GW], g[:, 3 * GW:4 * GW],
                                             AF.Sigmoid)
                        t1 = egp.tile([128, GW], F32, tag=f"t1{d}", name=f"t1{d}")
                        nc.vector.tensor_tensor(t1[:], g[:, GW:2 * GW], cst[:], ALU.mult)
                        t2 = egp.tile([128, GW], F32, tag=f"t2{d}", name=f"t2{d}")
                        nc.vector.tensor_tensor(t2[:], g[:, 0:GW], g[:, 2 * GW:3 * GW],
                                                ALU.mult)
                        nc.vector.tensor_tensor(cst[:], t1[:], t2[:], ALU.add)
                        tc_t = egp.tile([128, GW], F32, tag=f"tc{d}", name=f"tc{d}")
                        nc.scalar.activation(tc_t[:], cst[:], AF.Tanh)
                        nc.vector.tensor_tensor(hnxt[:], g[:, 3 * GW:4 * GW], tc_t[:],
                                                ALU.mult)
                        # store h into mem_T (cols (kk0+k)*B*S + b*S + s_mem)
                        nc.vector.tensor_copy(
                            memT4[:, kk0:kk0 + KE, :, s_mem],
                            hnxt[:].rearrange("p (k b) -> p k b", k=KE))

            if phases < 3:
                raise _Stop
            # ---------- c_t / h_t decoder init ----------
            h_d = [spool.tile([128, KD * B], BF16, tag=f"h_d{j}", name=f"h_d{j}") for j in range(2)]
            c_d = spool.tile([128, KD * B], F32, tag="c_d", name="c_d")
            # ccT = [cf.T; cb.T] as bf16 rhs chunks [128, B] (k = 0..KD-1)
            ccT = spool.tile([128, KD * B], BF16, tag="ccT", name="ccT")
            nc.vector.tensor_copy(ccT[:, 0:KE * B], c_f[:])
            nc.vector.tensor_copy(ccT[:, KE * B:2 * KE * B], c_b[:])
            with tc.tile_pool(name="ct_ps", bufs=2, space="PSUM") as cps, \
                 tc.tile_pool(name="ct_sb", bufs=2) as csb:
                for m in range(KD):
                    ps = cps.tile([128, B], F32, tag="ct", name="ct")
                    for k in range(KD):
                        lt = wtr_s[:, (k * KD + m) * 128:(k * KD + m + 1) * 128]
                        nc.tensor.matmul(ps[:], lt, ccT[:, k * B:(k + 1) * B],
                                         start=(k == 0), stop=(k == KD - 1))
                    # lrelu(y) = 0.55 y + 0.45 |y|
                    ab = csb.tile([128, B], F32, tag="ab", name="ab")
                    nc.scalar.activation(ab[:], ps[:], AF.Abs)
                    ident_t = csb.tile([128, B], F32, tag="idt", name="idt")
                    nc.scalar.activation(ident_t[:], ps[:], AF.Identity, scale=0.55)
                    nc.vector.scalar_tensor_tensor(c_d[:, m * B:(m + 1) * B], ab[:],
                                                   0.45, ident_t[:], ALU.mult, ALU.add)
                # h_t = s0 + lab*(s1-s0)
                for k in range(KD):
                    ps = cps.tile([128, B], F32, tag="ct", name="ct")
                    nc.tensor.matmul(ps[:], diff_s_s[:, k * 128:(k + 1) * 128],
                                     lab_d_s[:, :], start=True, stop=True)
                    f32t = csb.tile([128, B], F32, tag="h0d", name="h0d")
                    nc.vector.tensor_scalar_add(f32t[:], ps[:], s0T_s[:, k:k + 1])
                    nc.vector.tensor_copy(h_d[0][:, k * B:(k + 1) * B], f32t[:])

            if phases < 4:
                raise _Stop
            # ---------- decoder recurrence ----------
            H_T = big.tile([128, KD * B * T], BF16, tag="H_T", name="H_T")  # cols k*(B*T)+b*T+t
            HT4 = H_T[:].rearrange("p (k b t) -> p k b t", k=KD, b=B)
            with tc.tile_pool(name="dec_ps", bufs=2, space="PSUM") as dps, \
                 tc.tile_pool(name="dec_x", bufs=4) as dxp, \
                 tc.tile_pool(name="dec_g", bufs=3) as dgp:
                GW = KD * 32
                for step in range(T):
                    hcur = h_d[step % 2]
                    hnxt = h_d[(step + 1) % 2]
                    xt = dxp.tile([128, MD * 32], BF16, tag="xd", name="xd")
                    nc.sync.dma_start(xt[:], xd_d[step, :, :])
                    ps = dps.tile([128, MD * 32], F32, tag="gd", name="gd")
                    for m in range(MD):
                        for k in range(KD):
                            rhs = hcur[:, k * B:(k + 1) * B]
                            lt = whh_d_s[:, (k * MD + m) * 128:(k * MD + m + 1) * 128]
                            nc.tensor.matmul(ps[:, m * 32:(m + 1) * 32], lt, rhs,
                                             start=(k == 0), stop=(k == KD - 1))
                    g = dgp.tile([128, MD * 32], F32, tag="ggd", name="ggd")
                    nc.vector.tensor_tensor(g[:], ps[:], xt[:], ALU.add)
                    nc.scalar.activation(g[:, 0:2 * GW], g[:, 0:2 * GW], AF.Sigmoid)
                    nc.scalar.activation(g[:, 2 * GW:3 * GW], g[:, 2 * GW:3 * GW], AF.Tanh)
                    nc.scalar.activation(g[:, 3 * GW:4 * GW], g[:, 3 * GW:4 * GW],
                                         AF.Sigmoid)
                    t1 = dgp.tile([128, GW], F32, tag="t1d", name="t1d")
                    nc.vector.tensor_tensor(t1[:], g[:, GW:2 * GW], c_d[:], ALU.mult)
                    t2 = dgp.tile([128, GW], F32, tag="t2d", name="t2d")
                    nc.vector.tensor_tensor(t2[:], g[:, 0:GW], g[:, 2 * GW:3 * GW],
                                            ALU.mult)
                    nc.vector.tensor_tensor(c_d[:], t1[:], t2[:], ALU.add)
                    tc_t = dgp.tile([128, GW], F32, tag="tcd", name="tcd")
                    nc.scalar.activation(tc_t[:], c_d[:], AF.Tanh)
                    nc.vector.tensor_tensor(hnxt[:], g[:, 3 * GW:4 * GW], tc_t[:],
                                            ALU.mult)
                    nc.vector.tensor_copy(
                        HT4[:, :, :, step],
                        hnxt[:].rearrange("p (k b) -> p k b", k=KD))

            if phases < 5:
                raise _Stop
            # ---------- attention + FFN mid ----------
            mid_T = big.tile([128, KD * B * T], BF16, tag="mid_T", name="mid_T")  # cols k*(B*T)+bt
            BT = B * T
            n_mblk = BT // 512
            bpb = 512 // T if T <= 512 else 1  # b's per 512-col block
            with tc.tile_pool(name="at_ps", bufs=2, space="PSUM") as aps, \
                 tc.tile_pool(name="pt_ps", bufs=2, space="PSUM") as pps, \
                 tc.tile_pool(name="cx_ps", bufs=2, space="PSUM") as cps2, \
                 tc.tile_pool(name="md_ps", bufs=2, space="PSUM") as mps, \
                 tc.tile_pool(name="at_sb", bufs=3) as asb, \
                 tc.tile_pool(name="cx_sb", bufs=2) as cxs, \
                 tc.tile_pool(name="mn_sb", bufs=2) as mns:
                for blk in range(n_mblk):
                    ctx_blk = cxs.tile([128, KD, 512], BF16, tag="cxb", name="cxb")
                    for bi in range(bpb):
                        b = blk * bpb + bi
                        # scores A_b [T, S]
                        a_ps = aps.tile([T, S], F32, tag="a", name="a")
                        for k in range(2 * KE):
                            nc.tensor.matmul(a_ps[:], HT4[:, k, b, :],
                                             memT4[:, k, b, :],
                                             start=(k == 0), stop=(k == 2 * KE - 1))
                        mx = asb.tile([T, 1], F32, tag="mx", name="mx")
                        nc.vector.tensor_reduce(mx[:], a_ps[:], AX.X, ALU.max,
                                                negate=True)
                        mx2 = asb.tile([T, 1], F32, tag="mx2", name="mx2")
                        nc.scalar.mul(mx2[:], mx[:], SCALE)
                        ex = asb.tile([T, S], F32, tag="ex", name="ex")
                        den = asb.tile([T, 1], F32, tag="den", name="den")
                        nc.scalar.activation(ex[:], a_ps[:], AF.Exp, bias=mx2[:],
                                             scale=SCALE, accum_out=den[:])
                        rec = asb.tile([T, 1], F32, tag="rec", name="rec")
                        nc.vector.reciprocal(rec[:], den[:])
                        p_sb = asb.tile([T, S], BF16, tag="p", name="p")
                        nc.vector.tensor_scalar_mul(p_sb[:], ex[:], rec[:])
                        # P.T [S, T]
                        pt_ps = pps.tile([S, T], BF16, tag="pt", name="pt", padded_shape=[128, 128])
                        nc.tensor.transpose(pt_ps[:], p_sb[:], ident[0:T, 0:T])
                        pt_sb = asb.tile([S, T], BF16, tag="pts", name="pts")
                        nc.scalar.copy(pt_sb[:], pt_ps[:])
                        # mem_norm(b) [s, d] built on the fly by PE transpose
                        memNb = mns.tile([S, 2 * KE, 128], BF16, tag="mnb", name="mnb")
                        for kd in range(2 * KE):
                            mn_ps = pps.tile([S, 128], BF16, tag="pt", name="mnp", padded_shape=[128, 128])
                            nc.tensor.transpose(mn_ps[:], memT4[:, kd, b, :], ident[:])
                            nc.scalar.copy(memNb[:, kd, :], mn_ps[:])
                        # ctx.T chunks [128, T] = mem_norm(b,k).T @ P.T
                        for kd in range(KD):
                            c_ps = cps2.tile([128, T], F32, tag="c", name="c")
                            nc.tensor.matmul(c_ps[:], memNb[:, kd, :], pt_sb[:],
                                             start=True, stop=True)
                            nc.vector.tensor_copy(
                                ctx_blk[:, kd, bi * T:(bi + 1) * T], c_ps[:])
                    # mid.T [512-rows, this 512-col block]
                    for m in range(KD):
                        ps = mps.tile([128, 512], F32, tag="md", name="md")
                        for k in range(KD):  # h part
                            lt = wf1_s[:, (k * KD + m) * 128:(k * KD + m + 1) * 128]
                            nc.tensor.matmul(ps[:], lt,
                                             H_T[:, k * BT + blk * 512:
                                                 k * BT + (blk + 1) * 512],
                                             start=(k == 0), stop=False)
                        for k in range(KD):  # ctx part
                            kk = KD + k
                            lt = wf1_s[:, (kk * KD + m) * 128:(kk * KD + m + 1) * 128]
                            nc.tensor.matmul(ps[:], lt, ctx_blk[:, k, :],
                                             start=False, stop=(k == KD - 1))
                        ab = asb.tile([128, 512], F32, tag="mab", name="mab")
                        nc.scalar.activation(ab[:], ps[:], AF.Abs,
                                             bias=b1a_s[:, m:m + 1])
                        idt = asb.tile([128, 512], F32, tag="mid", name="mid")
                        nc.scalar.activation(idt[:], ps[:], AF.Identity, scale=0.55,
                                             bias=b1h_s[:, m:m + 1])
                        nc.vector.scalar_tensor_tensor(
                            mid_T[:, m * BT + blk * 512:m * BT + (blk + 1) * 512],
                            ab[:], 0.45, idt[:], ALU.mult, ALU.add)

            if phases < 6:
                raise _Stop
            # ---------- vocab projection ----------
            wf2_3d = wf2.ap().rearrange("p (k v) -> p k v", k=KD)
            with tc.tile_pool(name="lg_ps", bufs=2, space="PSUM") as lps, \
                 tc.tile_pool(name="lg_sb", bufs=3) as lsb, \
                 tc.tile_pool(name="wf2_sb", bufs=2) as wfp:
                for c in range(NVC):
                    w0 = c * 500
                    w1 = min(VS, w0 + 500)
                    nw = w1 - w0
                    wf2c = wfp.tile([128, KD, 500], BF16, tag="wf2c", name="wf2c")
                    nc.sync.dma_start(wf2c[:, :, 0:nw], wf2_3d[:, :, w0:w1])
                    for btm in range(BT // 128):
                        ps = lps.tile([128, 500], F32, tag="lg", name="lg")
                        for k in range(KD):
                            lt = mid_T[:, k * BT + btm * 128:k * BT + (btm + 1) * 128]
                            nc.tensor.matmul(ps[:, 0:nw], lt,
                                             wf2c[:, k, 0:nw],
                                             start=(k == 0), stop=(k == KD - 1))
                        sb = lsb.tile([128, 500], F32, tag="lo", name="lo")
                        nc.scalar.copy(sb[:, 0:nw], ps[:, 0:nw])
                        # btm-th 128 bt-cols are b = btm*128//T .. : with b-major
                        # cols b*T+t, a 128-col chunk spans 128/T b's (T>=128: b=btm)
                        if T >= 128:
                            b0 = (btm * 128) // T
                            t0 = (btm * 128) % T
                            nc.sync.dma_start(out.ap()[b0, t0:t0 + 128, w0:w1],
                                              sb[:, 0:nw])
                        else:
                            nb = 128 // T
                            b0 = btm * nb
                            nc.sync.dma_start(
                                out.ap()[b0:b0 + nb, :, w0:w1]
                                .rearrange("b t v -> (b t) v"),
                                sb[:, 0:nw])

        except _Stop:
            pass
    nc.compile()
    return nc


def prep_inputs(i, S=128, T=128, V=32000, VS=4000):
    """Host-side input staging -> list of 8 per-core in_maps."""
    KD = D_DEC // 128
    ME = 4 * D_ENC // 128
    MD = 4 * D_DEC // 128
    KE = D_ENC // 128

    def as_np(x, dt=np.float32):
        return np.ascontiguousarray(np.asarray(x), dtype=dt)

    tok = as_np(i["tok_emb"]).astype(bf16)

    def idx_prep(flat):
        a = flat.astype(np.int16).reshape(-1, 16).T  # [16, N/16]
        return np.ascontiguousarray(np.tile(a, (8, 1)))

    inp = as_np(i["inp"], np.int64)
    x = as_np(i["x"], np.int64)
    idx_e = idx_prep(inp.T.reshape(-1))              # s-major: s*32+b
    dmat = np.zeros((B, T), np.int64)
    dmat[:, 1:] = x[:, :T - 1]
    idx_d = idx_prep(dmat.T.reshape(-1))             # t-major: t*32+b

    startT = as_np(i["start_emb"]).reshape(D_EMB, 1).astype(bf16)

    est = as_np(i["enc_style_emb"])                  # [2, 512]
    diff_e = (est[1] - est[0]).reshape(1, -1).astype(bf16)
    e0T = np.ascontiguousarray(est[0].reshape(KD, 128).T)  # [128, 4]
    sty = as_np(i["style_emb"])                      # [2, 512]
    diff_s = (sty[1] - sty[0]).reshape(1, -1).astype(bf16)
    s0T = np.ascontiguousarray(sty[0].reshape(KD, 128).T)
    lab_i = as_np(i["label_i"], np.float32).reshape(1, B).astype(bf16)
    lab_d = as_np(i["label"], np.float32).reshape(1, B).astype(bf16)

    def wihT(w, nm):   # w [4H, 128] -> [128, nm*128]
        a = w.reshape(nm, 128, 128)        # [m, c, p]
        return np.ascontiguousarray(a.transpose(2, 0, 1).reshape(128, nm * 128)
                                    ).astype(bf16)

    def whhT(w, nk, nm):  # w [4H, H] -> [128, nk*nm*128], (k,m)-major
        a = w.reshape(nm, 128, nk, 128)    # [m, c, k, p]
        a = a.transpose(3, 2, 0, 1)        # [p, k, m, c]
        return np.ascontiguousarray(a.reshape(128, nk * nm * 128)).astype(bf16)

    wih_f = wihT(as_np(i["Wih_f"]), ME)
    wih_b = wihT(as_np(i["Wih_b"]), ME)
    wih_d = wihT(as_np(i["Wih_d"]), MD)
    whh_f = whhT(as_np(i["Whh_f"]), KE, ME)
    whh_b = whhT(as_np(i["Whh_b"]), KE, ME)
    whh_d = whhT(as_np(i["Whh_d"]), KD, MD)
    wtr = whhT(as_np(i["W_tr"]), KD, KD)
    wf1 = whhT(as_np(i["W_f1"]), 8, KD)

    wf2_full = as_np(i["W_f2"])                      # [V, 512]

    def bcol(v, nm):
        return np.ascontiguousarray(v.reshape(nm, 128).T)  # [128, nm]

    bs_f = bcol(as_np(i["bih_f"]) + as_np(i["bhh_f"]), ME)
    bs_b = bcol(as_np(i["bih_b"]) + as_np(i["bhh_b"]), ME)
    bs_d = bcol(as_np(i["bih_d"]) + as_np(i["bhh_d"]), MD)
    b1 = as_np(i["b_f1"])
    b1a = bcol(b1, KD)
    b1h = bcol(0.55 * b1, KD)

    common = dict(tokb=tok, idx_e=idx_e, idx_d=idx_d, startT=startT,
                  diff_e=diff_e, e0T=e0T, lab_i=lab_i,
                  diff_s=diff_s, s0T=s0T, lab_d=lab_d,
                  wih_f=wih_f, wih_b=wih_b, wih_d=wih_d,
                  whh_f=whh_f, whh_b=whh_b, whh_d=whh_d,
                  wtr=wtr, wf1=wf1,
                  bs_f=bs_f, bs_b=bs_b, bs_d=bs_d, b1a=b1a, b1h=b1h)
    in_maps = []
    for c in range(N_CORES):
        shard = wf2_full[c * VS:(c + 1) * VS]        # [VS, 512]
        a = shard.reshape(VS, KD, 128)               # [v, k, p]
        wf2 = np.ascontiguousarray(a.transpose(2, 1, 0).reshape(128, KD * VS)
                                   ).astype(bf16)
        in_maps.append(dict(common, wf2=wf2))
    return in_maps


_NC_CACHE = {}


def kernel(**inputs):
    key = "full"
    if key not in _NC_CACHE:
        _NC_CACHE[key] = build()
    nc = _NC_CACHE[key]
    in_maps = prep_inputs(inputs)
    res = run_bass_kernel_spmd(nc, in_maps, core_ids=list(range(N_CORES)))
    return np.concatenate([r["out"] for r in res.results], axis=2)

